# revision 24
# baseline (speedup 1.0000x reference)
"""Multi-head attention (B=4, S=2048, D=1024, H=16) on 8 trn2 NeuronCores.

Sharding: (batch x head-half) -> 8 shards, tensor-parallel over heads.
Core c handles batch b=c//2 and heads hh*8..hh*8+8 (hh=c%2), computing a
partial output projection over its 512 contraction dims; the pairwise
all-reduce of the output projection (and the bo add) happens host-side
during unshard.

Numerics: the q/k/v projections run as fp8e4m3 DoubleRow matmuls with
both operands error-compensated (x = xh+xl, W = 32*(Wh+Wl), hi/lo split
done on the host), which costs 12 DR instructions per 8-chunk
contraction instead of 8 bf16 ones at 2x the rate -> 1.33x faster with
bf16-level accuracy. The 32x weight scale keeps W's hi/lo split out of
fp8's subnormal range; q/k carry the 32x into the scores, folded into
the exp scale (0.125/32/32), and v's 32x is cancelled by setting the
softmax-normalizer column of v_aug to 32 instead of 1.

Attention (scores, exp, es@v) stays bf16: fp8 anywhere on that path
measures ~5e-2 max-rel error (vs the 2e-2 gate) unless both sides are
compensated, which erases the speedup.

Schedule: minimal serial prefix (Q fc0 half, K fc0 first chunk), then
the attention loop starts at ~10us. The V projection is computed
per-head just-in-time (each qh=0 head emits the 12 DR matmuls for its
own 64 v columns at each kc), spreading V's PE cost evenly instead of
front-loading it; remaining q/k/o blocks ride as ~2 filler blocks per
head. Per kc the emission order is sc -> exp -> pv -> fillers so the
Activation engine (the secondary bottleneck, 256 x 1us exp) is fed
first. Softmax normalizer broadcast runs on the idle GpSimd engine;
PSUM drains run on DVE during attention and on Act in the tail.
"""
import sys
sys.path.insert(0, '/opt/trn_rl_repo')
import numpy as np
import ml_dtypes
import concourse.bass as bass
from concourse import bacc
import concourse.mybir as mybir
import concourse.tile as tile
from concourse.bass_utils import run_bass_kernel_spmd

dt = mybir.dt
F = mybir.ActivationFunctionType
DR = mybir.MatmulPerfMode.DoubleRow

B, S, D, H = 4, 2048, 1024, 16
DK = D // H          # 64
NC = 8               # cores
HC = 8               # heads per core
FW = 512             # feature width per core (HC*DK)
FC = FW // 128       # 4 feature chunks
DC = D // 128        # 8 input d-chunks
KC = S // 128        # 16 key chunks
G = 65               # v_aug group width (64 v cols + norm col)
WS = 32.0            # weight pre-scale for fp8 hi/lo representability
EXP_SCALE = 0.125 / (WS * WS)

_nc_cache = None


def build_nc():
    nc = bacc.Bacc()
    bf16 = dt.bfloat16
    f8 = dt.float8e4
    qT_h = nc.dram_tensor("qT_h", [D, S], f8, kind="ExternalInput")
    qT_l = nc.dram_tensor("qT_l", [D, S], f8, kind="ExternalInput")
    kT_h = nc.dram_tensor("kT_h", [D, S], f8, kind="ExternalInput")
    kT_l = nc.dram_tensor("kT_l", [D, S], f8, kind="ExternalInput")
    vT_h = nc.dram_tensor("vT_h", [D, S], f8, kind="ExternalInput")
    vT_l = nc.dram_tensor("vT_l", [D, S], f8, kind="ExternalInput")
    WqT_h = nc.dram_tensor("WqT_h", [D, FW], f8, kind="ExternalInput")
    WqT_l = nc.dram_tensor("WqT_l", [D, FW], f8, kind="ExternalInput")
    WkT_h = nc.dram_tensor("WkT_h", [D, FW], f8, kind="ExternalInput")
    WkT_l = nc.dram_tensor("WkT_l", [D, FW], f8, kind="ExternalInput")
    WvT_h = nc.dram_tensor("WvT_h", [D, FW], f8, kind="ExternalInput")
    WvT_l = nc.dram_tensor("WvT_l", [D, FW], f8, kind="ExternalInput")
    WoR = nc.dram_tensor("WoR", [128, 2, 2, D], bf16, kind="ExternalInput")
    bq_pf = nc.dram_tensor("bq_pf", [128, FC], dt.float32, kind="ExternalInput")
    bk_pf = nc.dram_tensor("bk_pf", [128, FC], dt.float32, kind="ExternalInput")
    y_out = nc.dram_tensor("y_out", [S, D], dt.float32, kind="ExternalOutput")

    with tile.TileContext(nc) as tc:
        with tc.tile_pool(name="persist", bufs=1) as persist, \
             tc.tile_pool(name="pw", bufs=1) as pw, \
             tc.tile_pool(name="pin", bufs=1) as pin, \
             tc.tile_pool(name="pho2", bufs=2) as pho2:
            qT_sb = persist.tile([128, FC, S], bf16)         # 16 KB/part
            kT_sb = persist.tile([128, FC, S], bf16)         # 16 KB/part
            vaug_sb = persist.tile([128, KC, HC * G], bf16)  # 16.6 KB/part
            xh_sb = persist.tile([128, FC, S], bf16)         # 16 KB/part

            wqh = pw.tile([128, DC, FW], f8)
            wql = pw.tile([128, DC, FW], f8)
            wkh = pw.tile([128, DC, FW], f8)
            wkl = pw.tile([128, DC, FW], f8)
            wvh = pw.tile([128, DC, FW], f8)
            wvl = pw.tile([128, DC, FW], f8)
            bq_sb = pw.tile([128, FC], dt.float32)
            bk_sb = pw.tile([128, FC], dt.float32)

            def rrg(t):
                return t[:, :].rearrange("(c p) s -> p c s", p=128)

            qvh, qvl = rrg(qT_h), rrg(qT_l)
            kvh, kvl = rrg(kT_h), rrg(kT_l)
            vvh, vvl = rrg(vT_h), rrg(vT_l)

            qinh = [pin.tile([128, DC, 512], f8, tag="qinh", bufs=4, name=f"qinh{i}")
                    for i in range(4)]
            qinl = [pin.tile([128, DC, 512], f8, tag="qinl", bufs=4, name=f"qinl{i}")
                    for i in range(4)]
            kinh = [pin.tile([128, DC, 512], f8, tag="kinh", bufs=4, name=f"kinh{i}")
                    for i in range(4)]
            kinl = [pin.tile([128, DC, 512], f8, tag="kinl", bufs=4, name=f"kinl{i}")
                    for i in range(4)]
            vinh = [pin.tile([128, DC, 512], f8, tag="vinh", bufs=4, name=f"vinh{i}")
                    for i in range(4)]
            vinl = [pin.tile([128, DC, 512], f8, tag="vinl", bufs=4, name=f"vinl{i}")
                    for i in range(4)]

            # DMA issue order = urgency order (hi halves first: the first 4
            # DR matmuls of every projection block touch only hi tensors).
            wqvh = WqT_h[:, :].rearrange("(c p) f -> p c f", p=128)
            wqvl = WqT_l[:, :].rearrange("(c p) f -> p c f", p=128)
            wkvh = WkT_h[:, :].rearrange("(c p) f -> p c f", p=128)
            wkvl = WkT_l[:, :].rearrange("(c p) f -> p c f", p=128)
            nc.sync.dma_start(out=wqh[:, :, 0:128], in_=wqvh[:, :, 0:128])
            nc.scalar.dma_start(out=wql[:, :, 0:128], in_=wqvl[:, :, 0:128])
            nc.sync.dma_start(out=qinh[0], in_=qvh[:, :, 0:512])
            nc.scalar.dma_start(out=qinl[0], in_=qvl[:, :, 0:512])
            nc.sync.dma_start(out=qinh[1], in_=qvh[:, :, 512:1024])
            nc.scalar.dma_start(out=qinl[1], in_=qvl[:, :, 512:1024])
            nc.sync.dma_start(out=wkh[:, :, 0:128], in_=wkvh[:, :, 0:128])
            nc.scalar.dma_start(out=wkl[:, :, 0:128], in_=wkvl[:, :, 0:128])
            nc.sync.dma_start(out=kinh[0], in_=kvh[:, :, 0:512])
            nc.scalar.dma_start(out=kinl[0], in_=kvl[:, :, 0:512])
            nc.scalar.dma_start(out=bq_sb, in_=bq_pf[:, :])
            nc.scalar.dma_start(out=bk_sb, in_=bk_pf[:, :])
            nc.sync.dma_start(out=wvh, in_=WvT_h[:, :].rearrange("(c p) f -> p c f", p=128))
            nc.scalar.dma_start(out=wvl, in_=WvT_l[:, :].rearrange("(c p) f -> p c f", p=128))
            nc.sync.dma_start(out=vinh[0], in_=vvh[:, :, 0:512])
            nc.scalar.dma_start(out=vinl[0], in_=vvl[:, :, 0:512])
            nc.sync.dma_start(out=kinh[1], in_=kvh[:, :, 512:1024])
            nc.scalar.dma_start(out=kinl[1], in_=kvl[:, :, 512:1024])
            nc.sync.dma_start(out=vinh[1], in_=vvh[:, :, 512:1024])
            nc.scalar.dma_start(out=vinl[1], in_=vvl[:, :, 512:1024])
            nc.sync.dma_start(out=kinh[2], in_=kvh[:, :, 1024:1536])
            nc.scalar.dma_start(out=kinl[2], in_=kvl[:, :, 1024:1536])
            nc.sync.dma_start(out=vinh[2], in_=vvh[:, :, 1024:1536])
            nc.scalar.dma_start(out=vinl[2], in_=vvl[:, :, 1024:1536])
            nc.sync.dma_start(out=kinh[3], in_=kvh[:, :, 1536:2048])
            nc.scalar.dma_start(out=kinl[3], in_=kvl[:, :, 1536:2048])
            nc.sync.dma_start(out=vinh[3], in_=vvh[:, :, 1536:2048])
            nc.scalar.dma_start(out=vinl[3], in_=vvl[:, :, 1536:2048])
            nc.sync.dma_start(out=wqh[:, :, 128:FW], in_=wqvh[:, :, 128:FW])
            nc.scalar.dma_start(out=wql[:, :, 128:FW], in_=wqvl[:, :, 128:FW])
            nc.sync.dma_start(out=wkh[:, :, 128:FW], in_=wkvh[:, :, 128:FW])
            nc.scalar.dma_start(out=wkl[:, :, 128:FW], in_=wkvl[:, :, 128:FW])
            nc.sync.dma_start(out=qinh[2], in_=qvh[:, :, 1024:1536])
            nc.scalar.dma_start(out=qinl[2], in_=qvl[:, :, 1024:1536])
            nc.sync.dma_start(out=qinh[3], in_=qvh[:, :, 1536:2048])
            nc.scalar.dma_start(out=qinl[3], in_=qvl[:, :, 1536:2048])
            # Wo reuses two drained kin-hi buffers (tag rotation); its DMA
            # waits on the last k-projection reads automatically.
            wo_a = pin.tile([128, 2, D], bf16, tag="kinh", bufs=4, name="wo_a")
            wo_b = pin.tile([128, 2, D], bf16, tag="kinh", bufs=4, name="wo_b")
            nc.sync.dma_start(out=wo_a, in_=WoR[:, 0, :, :])
            nc.sync.dma_start(out=wo_b, in_=WoR[:, 1, :, :])

            # ---- fp8 DoubleRow projection blocks (both-comp: 12 DR each) ----
            def dr_proj(ps, wh, wl, xh_, xl_, col):
                n = 0
                for (w, x) in ((wh, xh_), (wl, xh_), (wh, xl_)):
                    for i in range(4):
                        nc.tensor.matmul(out=ps, lhsT=w[:, 2 * i:2 * i + 2, col],
                                         rhs=x[:, 2 * i:2 * i + 2, :],
                                         start=(n == 0), stop=(n == 11),
                                         perf_mode=DR)
                        n += 1

            def q_block(fc, rb, pool, tag, via_act):
                ps = pool.tile([128, 512], dt.float32, tag=tag, name="psq")
                col = slice(fc * 128, (fc + 1) * 128)
                dr_proj(ps, wqh, wql, qinh[rb], qinl[rb], col)
                dst = qT_sb[:, fc, rb * 512:(rb + 1) * 512]
                if via_act:
                    nc.scalar.activation(out=dst, in_=ps, func=F.Identity,
                                         bias=bq_sb[:, fc:fc + 1], scale=1.0)
                else:
                    with nc.allow_low_precision(reason="bf16 projection"):
                        nc.vector.tensor_scalar(out=dst, in0=ps, scalar1=bq_sb[:, fc:fc + 1],
                                                scalar2=None, op0=mybir.AluOpType.add)

            def k_block(fc, kb, pool, tag, via_act):
                ps = pool.tile([128, 512], dt.float32, tag=tag, name="psk")
                col = slice(fc * 128, (fc + 1) * 128)
                dr_proj(ps, wkh, wkl, kinh[kb], kinl[kb], col)
                dst = kT_sb[:, fc, kb * 512:(kb + 1) * 512]
                if via_act:
                    nc.scalar.activation(out=dst, in_=ps, func=F.Identity,
                                         bias=bk_sb[:, fc:fc + 1], scale=1.0)
                else:
                    with nc.allow_low_precision(reason="bf16 projection"):
                        nc.vector.tensor_scalar(out=dst, in0=ps, scalar1=bk_sb[:, fc:fc + 1],
                                                scalar2=None, op0=mybir.AluOpType.add)

            vaug_g = vaug_sb.rearrange("p t (g c) -> p t g c", g=HC)

            def v_seg_h(ps, h, rt):
                # per-head V: head h's 64 columns of chunk rt, accumulated
                # into segment rt%4 of a shared 4-chunk PSUM bank group.
                vb, sub = rt // 4, rt % 4
                scol = slice(sub * 128, (sub + 1) * 128)
                wcol = slice(h * 64, (h + 1) * 64)
                n = 0
                for (x, w) in ((vinh[vb], wvh), (vinh[vb], wvl), (vinl[vb], wvh)):
                    for i in range(4):
                        # start only once per bank group (zero-region covers
                        # the whole bank); stop on the group's last matmul.
                        nc.tensor.matmul(out=ps[:, rt % 4, :],
                                         lhsT=x[:, 2 * i:2 * i + 2, scol],
                                         rhs=w[:, 2 * i:2 * i + 2, wcol],
                                         start=(rt % 4 == 0 and n == 0),
                                         stop=(rt % 4 == 3 and n == 11),
                                         perf_mode=DR)
                        n += 1

            def v_drain(ps, h, g):
                nc.vector.tensor_copy(out=vaug_g[:, 4 * g:4 * g + 4, h, 0:64],
                                      in_=ps)

            def o_block(qs, pool, tag, split_dma=False, tail=False):
                ysb = pho2.tile([128, D], dt.float32, tag="ysb", name="ysb")
                for fb in range(2):
                    ps = pool.tile([128, 512], dt.float32, tag=tag, name="psy")
                    for hp in range(FC):
                        wt = wo_a if hp < 2 else wo_b
                        nc.tensor.matmul(out=ps, lhsT=xh_sb[:, hp, qs * 128:(qs + 1) * 128],
                                         rhs=wt[:, hp % 2, fb * 512:(fb + 1) * 512],
                                         start=(hp == 0), stop=(hp == FC - 1))
                    dst = ysb[:, fb * 512:(fb + 1) * 512]
                    if tail and fb == 0:
                        nc.scalar.copy(out=dst, in_=ps)
                    else:
                        nc.vector.tensor_copy(out=dst, in_=ps)
                    if split_dma:
                        nc.sync.dma_start(out=y_out[qs * 128:(qs + 1) * 128, fb * 512:(fb + 1) * 512],
                                          in_=ysb[:, fb * 512:(fb + 1) * 512])
                if not split_dma:
                    nc.sync.dma_start(out=y_out[qs * 128:(qs + 1) * 128, :], in_=ysb)

            # ---- minimal serial prefix: Q fc0 (qh=0), K fc0 kb0 ----
            with tc.tile_pool(name="psp", bufs=4, space="PSUM") as psp:
                q_block(0, 0, psp, "psp", True)
                q_block(0, 1, psp, "psp", True)
                k_block(0, 0, psp, "psp", True)
                nc.vector.memset(vaug_g[:, :, :, 64:65], WS)

            # ---- PE filler schedule: (qh, h, kc) -> blocks after pv ----
            def qb(fc, rb):
                return lambda: q_block(fc, rb, ps_x, "px", False)

            def kb_(fc, kb):
                return lambda: k_block(fc, kb, ps_x, "px", False)

            def ob(qs):
                return lambda: o_block(qs, ps_x, "px")

            post = {}

            def put(qh, h, kc, blk):
                post.setdefault((qh, h, kc), []).append(blk)

            put(0, 0, 2, kb_(0, 1))
            put(0, 0, 6, kb_(0, 2))
            put(0, 0, 10, kb_(0, 3))
            put(0, 1, 3, qb(1, 0))
            put(0, 1, 7, qb(1, 1))
            put(0, 1, 11, kb_(1, 0))
            put(0, 2, 3, kb_(1, 1))
            put(0, 2, 7, kb_(1, 2))
            put(0, 2, 11, kb_(1, 3))
            put(0, 3, 3, qb(2, 0))
            put(0, 3, 7, qb(2, 1))
            put(0, 3, 11, kb_(2, 0))
            put(0, 4, 3, kb_(2, 1))
            put(0, 4, 7, kb_(2, 2))
            put(0, 4, 11, kb_(2, 3))
            put(0, 5, 3, qb(3, 0))
            put(0, 5, 7, qb(3, 1))
            put(0, 5, 11, kb_(3, 0))
            put(0, 6, 3, kb_(3, 1))
            put(0, 6, 7, kb_(3, 2))
            put(0, 6, 11, kb_(3, 3))
            put(0, 7, 3, qb(0, 2))
            put(0, 7, 7, qb(0, 3))
            put(0, 7, 11, qb(1, 2))
            put(1, 0, 3, qb(1, 3))
            put(1, 0, 7, qb(2, 2))
            put(1, 0, 11, ob(0))
            put(1, 1, 3, qb(2, 3))
            put(1, 1, 7, qb(3, 2))
            put(1, 1, 11, ob(1))
            put(1, 2, 3, qb(3, 3))
            put(1, 2, 7, ob(2))
            put(1, 2, 11, ob(3))
            put(1, 3, 7, ob(4))
            put(1, 4, 11, ob(5))
            put(1, 5, 11, ob(6))
            put(1, 6, 11, ob(7))

            # ---- attention: per (qh, head), softmax(qk*EXP_SCALE) @ v_aug ----
            # pv lags sc by LAG kc so the 4-chunk V bank-group drains (one
            # DVE copy per 4 chunks) land before their first pv consumer.
            LAG = 4
            with tc.tile_pool(name="pha_es", bufs=5) as pha_es, \
                 tc.tile_pool(name="pha_sm", bufs=2) as pha_sm, \
                 tc.tile_pool(name="ps_sc", bufs=2, space="PSUM") as ps_sc, \
                 tc.tile_pool(name="ps_pv", bufs=1, space="PSUM") as ps_pv, \
                 tc.tile_pool(name="ps_x", bufs=2, space="PSUM") as ps_x:
                for qh in range(2):
                    q0 = qh * 1024
                    for h in range(HC):
                        off = (h % 2) * 64
                        fc = h // 2
                        pvA = ps_pv.tile([65, 512], dt.float32, tag="pvA")
                        pvB = ps_pv.tile([65, 512], dt.float32, tag="pvB")
                        es_q = []
                        ps_vg = None
                        for kc in range(KC + LAG):
                            if kc < KC:
                                sc = ps_sc.tile([128, 1024], dt.float32, tag="sc")
                                for qs in range(2):
                                    nc.tensor.matmul(out=sc[:, qs * 512:(qs + 1) * 512],
                                                     lhsT=kT_sb[off:off + 64, fc, kc * 128:(kc + 1) * 128],
                                                     rhs=qT_sb[off:off + 64, fc, q0 + qs * 512:q0 + (qs + 1) * 512],
                                                     start=True, stop=True)
                                es = pha_es.tile([128, 1024], dt.bfloat16, tag="es")
                                nc.scalar.activation(out=es, in_=sc, func=F.Exp,
                                                     scale=EXP_SCALE)
                                es_q.append(es)
                            if kc >= LAG:
                                j = kc - LAG
                                esj = es_q[j]
                                nc.tensor.matmul(out=pvA, lhsT=vaug_sb[:, j, h * G:h * G + G],
                                                 rhs=esj[:, 0:512], start=(j == 0), stop=(j == KC - 1))
                                nc.tensor.matmul(out=pvB, lhsT=vaug_sb[:, j, h * G:h * G + G],
                                                 rhs=esj[:, 512:1024], start=(j == 0), stop=(j == KC - 1))
                            if qh == 0 and kc < KC:
                                if kc % 4 == 0:
                                    ps_vg = ps_x.tile([128, 4, 64], dt.float32,
                                                      tag="px", name="psvg")
                                v_seg_h(ps_vg, h, kc)
                                if kc % 4 == 3:
                                    v_drain(ps_vg, h, kc // 4)
                            for blk in post.get((qh, h, kc), ()):
                                blk()
                        for qs, pv in ((0, pvA), (1, pvB)):
                            pv_sb = pha_sm.tile([65, 512], dt.bfloat16, tag="pv_sb")
                            with nc.allow_low_precision(reason="bf16 attention context"):
                                nc.scalar.copy(out=pv_sb, in_=pv)
                            recip = pha_sm.tile([1, 512], dt.bfloat16, tag="recip", bufs=1)
                            with nc.allow_low_precision(reason="bf16 softmax normalizer"):
                                nc.vector.reciprocal(out=recip, in_=pv_sb[64:65, :])
                            bc_sb = pha_sm.tile([64, 512], dt.bfloat16, tag="bc_sb", bufs=2)
                            nc.gpsimd.partition_broadcast(bc_sb, recip)
                            with nc.allow_low_precision(reason="bf16 attention context"):
                                nc.gpsimd.tensor_mul(
                                    out=xh_sb[off:off + 64, fc, q0 + qs * 512:q0 + (qs + 1) * 512],
                                    in0=pv_sb[0:64, :], in1=bc_sb)

            # ---- tail: second half of the output projection ----
            # Act engine is idle here (exp done), so it does the PSUM drains.
            with tc.tile_pool(name="ps_y", bufs=4, space="PSUM") as ps_y:
                for qs in range(8, 16):
                    o_block(qs, ps_y, "psy", split_dma=True, tail=True)

    nc.finalize()
    return nc


def _get_nc():
    global _nc_cache
    if _nc_cache is None:
        _nc_cache = build_nc()
    return _nc_cache


def _hilo(x):
    """fp8e4m3 hi/lo decomposition of a fp32 array."""
    f8 = ml_dtypes.float8_e4m3
    hi = x.astype(f8)
    lo = (x - hi.astype(np.float32)).astype(f8)
    return np.ascontiguousarray(hi), np.ascontiguousarray(lo)


def kernel(query, key_, value, mask, Wq, bq, Wk, bk, Wv, bv, Wo, bo):
    bf16 = ml_dtypes.bfloat16
    query = np.asarray(query, dtype=np.float32)
    key_ = np.asarray(key_, dtype=np.float32)
    value = np.asarray(value, dtype=np.float32)
    Wq = np.asarray(Wq, dtype=np.float32)
    bq = np.asarray(bq, dtype=np.float32)
    Wk = np.asarray(Wk, dtype=np.float32)
    bk = np.asarray(bk, dtype=np.float32)
    Wv = np.asarray(Wv, dtype=np.float32)
    bv = np.asarray(bv, dtype=np.float32)
    Wo = np.asarray(Wo, dtype=np.float32)
    bo = np.asarray(bo, dtype=np.float32)

    nc = _get_nc()

    xT = {}
    for name, x in (("q", query), ("k", key_), ("v", value)):
        xT[name] = [_hilo(np.ascontiguousarray(x[b].T)) for b in range(B)]

    WqT = Wq.T * WS
    WkT = Wk.T * WS
    WvT = Wv.T * WS
    WoT = Wo.T
    halves = []
    for hh in range(2):
        cols = slice(hh * FW, (hh + 1) * FW)
        wq_h, wq_l = _hilo(np.ascontiguousarray(WqT[:, cols]))
        wk_h, wk_l = _hilo(np.ascontiguousarray(WkT[:, cols]))
        wv_h, wv_l = _hilo(np.ascontiguousarray(WvT[:, cols]))
        halves.append({
            "WqT_h": wq_h, "WqT_l": wq_l,
            "WkT_h": wk_h, "WkT_l": wk_l,
            "WvT_h": wv_h, "WvT_l": wv_l,
            # WoR[p, half, hp2, f] = Wo.T[hh*512 + (half*2+hp2)*128 + p, f]
            "WoR": np.ascontiguousarray(
                WoT[cols].reshape(2, 2, 128, D).transpose(2, 0, 1, 3).astype(bf16)),
            # q/k biases carry the WS weight scale (folded out in exp scale)
            "bq_pf": np.ascontiguousarray(bq[cols].reshape(FC, 128).T) * WS,
            "bk_pf": np.ascontiguousarray(bk[cols].reshape(FC, 128).T) * WS,
        })

    in_maps = []
    for c in range(NC):
        b, hh = c // 2, c % 2
        m = {
            "qT_h": xT["q"][b][0], "qT_l": xT["q"][b][1],
            "kT_h": xT["k"][b][0], "kT_l": xT["k"][b][1],
            "vT_h": xT["v"][b][0], "vT_l": xT["v"][b][1],
        }
        m.update(halves[hh])
        in_maps.append(m)

    res = run_bass_kernel_spmd(nc, in_maps, core_ids=list(range(NC)))

    # pairwise all-reduce of the tensor-parallel output projection (unshard).
    # bo and the bv contribution (bv @ Wo.T, constant across positions) are
    # folded into the host-side reduction.
    bias_row = bo + bv @ Wo.T
    y = np.empty((B, S, D), dtype=np.float32)
    for b in range(B):
        np.add(res.results[2 * b]["y_out"], res.results[2 * b + 1]["y_out"], out=y[b])
        y[b] += bias_row
    return y


# revision 25
# speedup vs baseline: 1.0507x; 1.0507x over previous
"""Multi-head attention (B=4, S=2048, D=1024, H=16) on 8 trn2 NeuronCores.

Sharding: (batch x head-half) -> 8 shards, tensor-parallel over heads.
Core c handles batch b=c//2 and heads hh*8..hh*8+8 (hh=c%2), computing a
partial output projection over its 512 contraction dims; the pairwise
all-reduce of the output projection (and the bo add) happens host-side
during unshard.

Numerics: the q/k/v projections run as fp8e4m3 DoubleRow matmuls with
both operands error-compensated (x = xh+xl, W = 32*(Wh+Wl), hi/lo split
done on the host), which costs 12 DR instructions per 8-chunk
contraction instead of 8 bf16 ones at 2x the rate -> 1.33x faster with
bf16-level accuracy. The 32x weight scale keeps W's hi/lo split out of
fp8's subnormal range; q/k carry the 32x into the scores, folded into
the exp scale (0.125/32/32), and v's 32x is cancelled by setting the
softmax-normalizer column of v_aug to 32 instead of 1.

Attention (scores, exp, es@v) stays bf16: fp8 anywhere on that path
measures ~5e-2 max-rel error (vs the 2e-2 gate) unless both sides are
compensated, which erases the speedup.

Schedule: minimal serial prefix (Q fc0 half, K fc0 first chunk), then
the attention loop starts at ~10us. The V projection is computed
per-head just-in-time (each qh=0 head emits the 12 DR matmuls for its
own 64 v columns at each kc), spreading V's PE cost evenly instead of
front-loading it; remaining q/k/o blocks ride as ~2 filler blocks per
head. Per kc the emission order is sc -> exp -> pv -> fillers so the
Activation engine (the secondary bottleneck, 256 x 1us exp) is fed
first. Softmax normalizer broadcast runs on the idle GpSimd engine;
PSUM drains run on DVE during attention and on Act in the tail.
"""
import sys
sys.path.insert(0, '/opt/trn_rl_repo')
import numpy as np
import ml_dtypes
import concourse.bass as bass
from concourse import bacc
import concourse.mybir as mybir
import concourse.tile as tile
from concourse.bass_utils import run_bass_kernel_spmd

dt = mybir.dt
F = mybir.ActivationFunctionType
DR = mybir.MatmulPerfMode.DoubleRow

B, S, D, H = 4, 2048, 1024, 16
DK = D // H          # 64
NC = 8               # cores
HC = 8               # heads per core
FW = 512             # feature width per core (HC*DK)
FC = FW // 128       # 4 feature chunks
DC = D // 128        # 8 input d-chunks
KC = S // 128        # 16 key chunks
G = 65               # v_aug group width (64 v cols + norm col)
WS = 32.0            # weight pre-scale for fp8 hi/lo representability
EXP_SCALE = 0.125 / (WS * WS)

_nc_cache = None


def build_nc():
    nc = bacc.Bacc()
    bf16 = dt.bfloat16
    f8 = dt.float8e4
    qT_h = nc.dram_tensor("qT_h", [D, S], f8, kind="ExternalInput")
    qT_l = nc.dram_tensor("qT_l", [D, S], f8, kind="ExternalInput")
    kT_h = nc.dram_tensor("kT_h", [D, S], f8, kind="ExternalInput")
    kT_l = nc.dram_tensor("kT_l", [D, S], f8, kind="ExternalInput")
    vT_h = nc.dram_tensor("vT_h", [D, S], f8, kind="ExternalInput")
    vT_l = nc.dram_tensor("vT_l", [D, S], f8, kind="ExternalInput")
    WqT_h = nc.dram_tensor("WqT_h", [D, FW], f8, kind="ExternalInput")
    WqT_l = nc.dram_tensor("WqT_l", [D, FW], f8, kind="ExternalInput")
    WkT_h = nc.dram_tensor("WkT_h", [D, FW], f8, kind="ExternalInput")
    WkT_l = nc.dram_tensor("WkT_l", [D, FW], f8, kind="ExternalInput")
    WvT_h = nc.dram_tensor("WvT_h", [D, FW], f8, kind="ExternalInput")
    WvT_l = nc.dram_tensor("WvT_l", [D, FW], f8, kind="ExternalInput")
    WoR = nc.dram_tensor("WoR", [128, 2, 2, D], bf16, kind="ExternalInput")
    bq_pf = nc.dram_tensor("bq_pf", [128, FC], dt.float32, kind="ExternalInput")
    bk_pf = nc.dram_tensor("bk_pf", [128, FC], dt.float32, kind="ExternalInput")
    y_out = nc.dram_tensor("y_out", [S, D], dt.float32, kind="ExternalOutput")

    with tile.TileContext(nc) as tc:
        with tc.tile_pool(name="persist", bufs=1) as persist, \
             tc.tile_pool(name="pw", bufs=1) as pw, \
             tc.tile_pool(name="pin", bufs=1) as pin, \
             tc.tile_pool(name="pho2", bufs=2) as pho2:
            qT_sb = persist.tile([128, FC, S], bf16)         # 16 KB/part
            kT_sb = persist.tile([128, FC, S], bf16)         # 16 KB/part
            vaug_sb = persist.tile([128, KC, HC * G], bf16)  # 16.6 KB/part
            xh_sb = persist.tile([128, FC, S], bf16)         # 16 KB/part

            wqh = pw.tile([128, DC, FW], f8)
            wql = pw.tile([128, DC, FW], f8)
            wkh = pw.tile([128, DC, FW], f8)
            wkl = pw.tile([128, DC, FW], f8)
            wvh = pw.tile([128, DC, FW], f8)
            wvl = pw.tile([128, DC, FW], f8)
            bq_sb = pw.tile([128, FC], dt.float32)
            bk_sb = pw.tile([128, FC], dt.float32)

            def rrg(t):
                return t[:, :].rearrange("(c p) s -> p c s", p=128)

            qvh, qvl = rrg(qT_h), rrg(qT_l)
            kvh, kvl = rrg(kT_h), rrg(kT_l)
            vvh, vvl = rrg(vT_h), rrg(vT_l)

            qinh = [pin.tile([128, DC, 512], f8, tag="qinh", bufs=4, name=f"qinh{i}")
                    for i in range(4)]
            qinl = [pin.tile([128, DC, 512], f8, tag="qinl", bufs=4, name=f"qinl{i}")
                    for i in range(4)]
            kinh = [pin.tile([128, DC, 512], f8, tag="kinh", bufs=4, name=f"kinh{i}")
                    for i in range(4)]
            kinl = [pin.tile([128, DC, 512], f8, tag="kinl", bufs=4, name=f"kinl{i}")
                    for i in range(4)]
            vinh = [pin.tile([128, DC, 512], f8, tag="vinh", bufs=4, name=f"vinh{i}")
                    for i in range(4)]
            vinl = [pin.tile([128, DC, 512], f8, tag="vinl", bufs=4, name=f"vinl{i}")
                    for i in range(4)]

            # DMA issue order = urgency order (hi halves first: the first 4
            # DR matmuls of every projection block touch only hi tensors).
            wqvh = WqT_h[:, :].rearrange("(c p) f -> p c f", p=128)
            wqvl = WqT_l[:, :].rearrange("(c p) f -> p c f", p=128)
            wkvh = WkT_h[:, :].rearrange("(c p) f -> p c f", p=128)
            wkvl = WkT_l[:, :].rearrange("(c p) f -> p c f", p=128)
            nc.sync.dma_start(out=wqh[:, :, 0:128], in_=wqvh[:, :, 0:128])
            nc.sync.dma_start(out=wql[:, :, 0:128], in_=wqvl[:, :, 0:128])
            nc.sync.dma_start(out=qinh[0], in_=qvh[:, :, 0:512])
            nc.sync.dma_start(out=qinl[0], in_=qvl[:, :, 0:512])
            nc.sync.dma_start(out=qinh[1], in_=qvh[:, :, 512:1024])
            nc.sync.dma_start(out=qinl[1], in_=qvl[:, :, 512:1024])
            nc.sync.dma_start(out=wkh[:, :, 0:128], in_=wkvh[:, :, 0:128])
            nc.sync.dma_start(out=wkl[:, :, 0:128], in_=wkvl[:, :, 0:128])
            nc.sync.dma_start(out=kinh[0], in_=kvh[:, :, 0:512])
            nc.sync.dma_start(out=kinl[0], in_=kvl[:, :, 0:512])
            nc.sync.dma_start(out=bq_sb, in_=bq_pf[:, :])
            nc.sync.dma_start(out=bk_sb, in_=bk_pf[:, :])
            nc.sync.dma_start(out=wvh, in_=WvT_h[:, :].rearrange("(c p) f -> p c f", p=128))
            nc.sync.dma_start(out=wvl, in_=WvT_l[:, :].rearrange("(c p) f -> p c f", p=128))
            nc.sync.dma_start(out=vinh[0], in_=vvh[:, :, 0:512])
            nc.sync.dma_start(out=vinl[0], in_=vvl[:, :, 0:512])
            nc.sync.dma_start(out=kinh[1], in_=kvh[:, :, 512:1024])
            nc.sync.dma_start(out=kinl[1], in_=kvl[:, :, 512:1024])
            nc.sync.dma_start(out=vinh[1], in_=vvh[:, :, 512:1024])
            nc.sync.dma_start(out=vinl[1], in_=vvl[:, :, 512:1024])
            nc.sync.dma_start(out=kinh[2], in_=kvh[:, :, 1024:1536])
            nc.sync.dma_start(out=kinl[2], in_=kvl[:, :, 1024:1536])
            nc.sync.dma_start(out=vinh[2], in_=vvh[:, :, 1024:1536])
            nc.sync.dma_start(out=vinl[2], in_=vvl[:, :, 1024:1536])
            nc.sync.dma_start(out=kinh[3], in_=kvh[:, :, 1536:2048])
            nc.sync.dma_start(out=kinl[3], in_=kvl[:, :, 1536:2048])
            nc.sync.dma_start(out=vinh[3], in_=vvh[:, :, 1536:2048])
            nc.sync.dma_start(out=vinl[3], in_=vvl[:, :, 1536:2048])
            nc.sync.dma_start(out=wqh[:, :, 128:FW], in_=wqvh[:, :, 128:FW])
            nc.sync.dma_start(out=wql[:, :, 128:FW], in_=wqvl[:, :, 128:FW])
            nc.sync.dma_start(out=wkh[:, :, 128:FW], in_=wkvh[:, :, 128:FW])
            nc.sync.dma_start(out=wkl[:, :, 128:FW], in_=wkvl[:, :, 128:FW])
            nc.sync.dma_start(out=qinh[2], in_=qvh[:, :, 1024:1536])
            nc.sync.dma_start(out=qinl[2], in_=qvl[:, :, 1024:1536])
            nc.sync.dma_start(out=qinh[3], in_=qvh[:, :, 1536:2048])
            nc.sync.dma_start(out=qinl[3], in_=qvl[:, :, 1536:2048])
            # Wo reuses two drained kin-hi buffers (tag rotation); its DMA
            # waits on the last k-projection reads automatically.
            wo_a = pin.tile([128, 2, D], bf16, tag="kinh", bufs=4, name="wo_a")
            wo_b = pin.tile([128, 2, D], bf16, tag="kinh", bufs=4, name="wo_b")
            nc.sync.dma_start(out=wo_a, in_=WoR[:, 0, :, :])
            nc.sync.dma_start(out=wo_b, in_=WoR[:, 1, :, :])

            # ---- fp8 DoubleRow projection blocks (both-comp: 12 DR each) ----
            def dr_proj(ps, wh, wl, xh_, xl_, col):
                n = 0
                for (w, x) in ((wh, xh_), (wl, xh_), (wh, xl_)):
                    for i in range(4):
                        nc.tensor.matmul(out=ps, lhsT=w[:, 2 * i:2 * i + 2, col],
                                         rhs=x[:, 2 * i:2 * i + 2, :],
                                         start=(n == 0), stop=(n == 11),
                                         perf_mode=DR)
                        n += 1

            def q_block(fc, rb, pool, tag, via_act):
                ps = pool.tile([128, 512], dt.float32, tag=tag, name="psq")
                col = slice(fc * 128, (fc + 1) * 128)
                dr_proj(ps, wqh, wql, qinh[rb], qinl[rb], col)
                dst = qT_sb[:, fc, rb * 512:(rb + 1) * 512]
                if via_act:
                    nc.scalar.activation(out=dst, in_=ps, func=F.Identity,
                                         bias=bq_sb[:, fc:fc + 1], scale=1.0)
                else:
                    with nc.allow_low_precision(reason="bf16 projection"):
                        nc.vector.tensor_scalar(out=dst, in0=ps, scalar1=bq_sb[:, fc:fc + 1],
                                                scalar2=None, op0=mybir.AluOpType.add)

            def k_block(fc, kb, pool, tag, via_act):
                ps = pool.tile([128, 512], dt.float32, tag=tag, name="psk")
                col = slice(fc * 128, (fc + 1) * 128)
                dr_proj(ps, wkh, wkl, kinh[kb], kinl[kb], col)
                dst = kT_sb[:, fc, kb * 512:(kb + 1) * 512]
                if via_act:
                    nc.scalar.activation(out=dst, in_=ps, func=F.Identity,
                                         bias=bk_sb[:, fc:fc + 1], scale=1.0)
                else:
                    with nc.allow_low_precision(reason="bf16 projection"):
                        nc.vector.tensor_scalar(out=dst, in0=ps, scalar1=bk_sb[:, fc:fc + 1],
                                                scalar2=None, op0=mybir.AluOpType.add)

            vaug_g = vaug_sb.rearrange("p t (g c) -> p t g c", g=HC)

            def v_seg_h(ps, h, rt):
                # per-head V: head h's 64 columns of chunk rt, accumulated
                # into segment rt%4 of a shared 4-chunk PSUM bank group.
                vb, sub = rt // 4, rt % 4
                scol = slice(sub * 128, (sub + 1) * 128)
                wcol = slice(h * 64, (h + 1) * 64)
                n = 0
                for (x, w) in ((vinh[vb], wvh), (vinh[vb], wvl), (vinl[vb], wvh)):
                    for i in range(4):
                        # start only once per bank group (zero-region covers
                        # the whole bank); stop on the group's last matmul.
                        nc.tensor.matmul(out=ps[:, rt % 4, :],
                                         lhsT=x[:, 2 * i:2 * i + 2, scol],
                                         rhs=w[:, 2 * i:2 * i + 2, wcol],
                                         start=(rt % 4 == 0 and n == 0),
                                         stop=(rt % 4 == 3 and n == 11),
                                         perf_mode=DR)
                        n += 1

            def v_drain(ps, h, g):
                nc.vector.tensor_copy(out=vaug_g[:, 4 * g:4 * g + 4, h, 0:64],
                                      in_=ps)

            def o_block(qs, pool, tag, split_dma=False, tail=False):
                ysb = pho2.tile([128, D], dt.float32, tag="ysb", name="ysb")
                for fb in range(2):
                    ps = pool.tile([128, 512], dt.float32, tag=tag, name="psy")
                    for hp in range(FC):
                        wt = wo_a if hp < 2 else wo_b
                        nc.tensor.matmul(out=ps, lhsT=xh_sb[:, hp, qs * 128:(qs + 1) * 128],
                                         rhs=wt[:, hp % 2, fb * 512:(fb + 1) * 512],
                                         start=(hp == 0), stop=(hp == FC - 1))
                    dst = ysb[:, fb * 512:(fb + 1) * 512]
                    if tail and fb == 0:
                        nc.scalar.copy(out=dst, in_=ps)
                    else:
                        nc.vector.tensor_copy(out=dst, in_=ps)
                    if split_dma:
                        nc.sync.dma_start(out=y_out[qs * 128:(qs + 1) * 128, fb * 512:(fb + 1) * 512],
                                          in_=ysb[:, fb * 512:(fb + 1) * 512])
                if not split_dma:
                    nc.sync.dma_start(out=y_out[qs * 128:(qs + 1) * 128, :], in_=ysb)

            # ---- minimal serial prefix: Q fc0 (qh=0), K fc0 kb0 ----
            with tc.tile_pool(name="psp", bufs=4, space="PSUM") as psp:
                q_block(0, 0, psp, "psp", True)
                q_block(0, 1, psp, "psp", True)
                k_block(0, 0, psp, "psp", True)
                nc.vector.memset(vaug_g[:, :, :, 64:65], WS)

            # ---- PE filler schedule: (qh, h, kc) -> blocks after pv ----
            def qb(fc, rb):
                return lambda: q_block(fc, rb, ps_x, "px", False)

            def kb_(fc, kb):
                return lambda: k_block(fc, kb, ps_x, "px", False)

            def ob(qs):
                return lambda: o_block(qs, ps_x, "px")

            post = {}

            def put(qh, h, kc, blk):
                post.setdefault((qh, h, kc), []).append(blk)

            put(0, 0, 2, kb_(0, 1))
            put(0, 0, 6, kb_(0, 2))
            put(0, 0, 10, kb_(0, 3))
            put(0, 1, 3, qb(1, 0))
            put(0, 1, 7, qb(1, 1))
            put(0, 1, 11, kb_(1, 0))
            put(0, 2, 3, kb_(1, 1))
            put(0, 2, 7, kb_(1, 2))
            put(0, 2, 11, kb_(1, 3))
            put(0, 3, 3, qb(2, 0))
            put(0, 3, 7, qb(2, 1))
            put(0, 3, 11, kb_(2, 0))
            put(0, 4, 3, kb_(2, 1))
            put(0, 4, 7, kb_(2, 2))
            put(0, 4, 11, kb_(2, 3))
            put(0, 5, 3, qb(3, 0))
            put(0, 5, 7, qb(3, 1))
            put(0, 5, 11, kb_(3, 0))
            put(0, 6, 3, kb_(3, 1))
            put(0, 6, 7, kb_(3, 2))
            put(0, 6, 11, kb_(3, 3))
            put(0, 7, 3, qb(0, 2))
            put(0, 7, 7, qb(0, 3))
            put(0, 7, 11, qb(1, 2))
            put(1, 0, 3, qb(1, 3))
            put(1, 0, 7, qb(2, 2))
            put(1, 0, 11, ob(0))
            put(1, 1, 3, qb(2, 3))
            put(1, 1, 7, qb(3, 2))
            put(1, 1, 11, ob(1))
            put(1, 2, 3, qb(3, 3))
            put(1, 2, 7, ob(2))
            put(1, 2, 11, ob(3))
            put(1, 3, 7, ob(4))
            put(1, 4, 11, ob(5))
            put(1, 5, 11, ob(6))
            put(1, 6, 11, ob(7))

            # ---- attention: per (qh, head), softmax(qk*EXP_SCALE) @ v_aug ----
            # pv lags sc by LAG kc so the 4-chunk V bank-group drains (one
            # DVE copy per 4 chunks) land before their first pv consumer.
            LAG = 4
            with tc.tile_pool(name="pha_es", bufs=5) as pha_es, \
                 tc.tile_pool(name="pha_sm", bufs=2) as pha_sm, \
                 tc.tile_pool(name="ps_sc", bufs=2, space="PSUM") as ps_sc, \
                 tc.tile_pool(name="ps_pv", bufs=1, space="PSUM") as ps_pv, \
                 tc.tile_pool(name="ps_x", bufs=2, space="PSUM") as ps_x:
                for qh in range(2):
                    q0 = qh * 1024
                    for h in range(HC):
                        off = (h % 2) * 64
                        fc = h // 2
                        pvA = ps_pv.tile([65, 512], dt.float32, tag="pvA")
                        pvB = ps_pv.tile([65, 512], dt.float32, tag="pvB")
                        es_q = []
                        ps_vg = None
                        for kc in range(KC + LAG):
                            if kc < KC:
                                sc = ps_sc.tile([128, 1024], dt.float32, tag="sc")
                                for qs in range(2):
                                    nc.tensor.matmul(out=sc[:, qs * 512:(qs + 1) * 512],
                                                     lhsT=kT_sb[off:off + 64, fc, kc * 128:(kc + 1) * 128],
                                                     rhs=qT_sb[off:off + 64, fc, q0 + qs * 512:q0 + (qs + 1) * 512],
                                                     start=True, stop=True)
                                es = pha_es.tile([128, 1024], dt.bfloat16, tag="es")
                                nc.scalar.activation(out=es, in_=sc, func=F.Exp,
                                                     scale=EXP_SCALE)
                                es_q.append(es)
                            if kc >= LAG:
                                j = kc - LAG
                                esj = es_q[j]
                                nc.tensor.matmul(out=pvA, lhsT=vaug_sb[:, j, h * G:h * G + G],
                                                 rhs=esj[:, 0:512], start=(j == 0), stop=(j == KC - 1))
                                nc.tensor.matmul(out=pvB, lhsT=vaug_sb[:, j, h * G:h * G + G],
                                                 rhs=esj[:, 512:1024], start=(j == 0), stop=(j == KC - 1))
                            if qh == 0 and kc < KC:
                                if kc % 4 == 0:
                                    ps_vg = ps_x.tile([128, 4, 64], dt.float32,
                                                      tag="px", name="psvg")
                                v_seg_h(ps_vg, h, kc)
                                if kc % 4 == 3:
                                    v_drain(ps_vg, h, kc // 4)
                            for blk in post.get((qh, h, kc), ()):
                                blk()
                        for qs, pv in ((0, pvA), (1, pvB)):
                            pv_sb = pha_sm.tile([65, 512], dt.bfloat16, tag="pv_sb")
                            with nc.allow_low_precision(reason="bf16 attention context"):
                                # Act has slack while qh=0 is PE-bound; in
                                # qh=1 Act paces the loop, so DVE drains.
                                if qh == 0:
                                    nc.scalar.copy(out=pv_sb, in_=pv)
                                else:
                                    nc.vector.tensor_copy(out=pv_sb, in_=pv)
                            recip = pha_sm.tile([1, 512], dt.bfloat16, tag="recip", bufs=1)
                            with nc.allow_low_precision(reason="bf16 softmax normalizer"):
                                nc.vector.reciprocal(out=recip, in_=pv_sb[64:65, :])
                            bc_sb = pha_sm.tile([64, 512], dt.bfloat16, tag="bc_sb", bufs=2)
                            nc.gpsimd.partition_broadcast(bc_sb, recip)
                            with nc.allow_low_precision(reason="bf16 attention context"):
                                nc.vector.tensor_mul(
                                    out=xh_sb[off:off + 64, fc, q0 + qs * 512:q0 + (qs + 1) * 512],
                                    in0=pv_sb[0:64, :], in1=bc_sb)

            # ---- tail: second half of the output projection ----
            # Act engine is idle here (exp done), so it does the PSUM drains.
            with tc.tile_pool(name="ps_y", bufs=4, space="PSUM") as ps_y:
                for qs in range(8, 16):
                    o_block(qs, ps_y, "psy", split_dma=True, tail=True)

    nc.finalize()
    return nc


def _get_nc():
    global _nc_cache
    if _nc_cache is None:
        _nc_cache = build_nc()
    return _nc_cache


def _hilo(x):
    """fp8e4m3 hi/lo decomposition of a fp32 array."""
    f8 = ml_dtypes.float8_e4m3
    hi = x.astype(f8)
    lo = (x - hi.astype(np.float32)).astype(f8)
    return np.ascontiguousarray(hi), np.ascontiguousarray(lo)


def kernel(query, key_, value, mask, Wq, bq, Wk, bk, Wv, bv, Wo, bo):
    bf16 = ml_dtypes.bfloat16
    query = np.asarray(query, dtype=np.float32)
    key_ = np.asarray(key_, dtype=np.float32)
    value = np.asarray(value, dtype=np.float32)
    Wq = np.asarray(Wq, dtype=np.float32)
    bq = np.asarray(bq, dtype=np.float32)
    Wk = np.asarray(Wk, dtype=np.float32)
    bk = np.asarray(bk, dtype=np.float32)
    Wv = np.asarray(Wv, dtype=np.float32)
    bv = np.asarray(bv, dtype=np.float32)
    Wo = np.asarray(Wo, dtype=np.float32)
    bo = np.asarray(bo, dtype=np.float32)

    nc = _get_nc()

    xT = {}
    for name, x in (("q", query), ("k", key_), ("v", value)):
        xT[name] = [_hilo(np.ascontiguousarray(x[b].T)) for b in range(B)]

    WqT = Wq.T * WS
    WkT = Wk.T * WS
    WvT = Wv.T * WS
    WoT = Wo.T
    halves = []
    for hh in range(2):
        cols = slice(hh * FW, (hh + 1) * FW)
        wq_h, wq_l = _hilo(np.ascontiguousarray(WqT[:, cols]))
        wk_h, wk_l = _hilo(np.ascontiguousarray(WkT[:, cols]))
        wv_h, wv_l = _hilo(np.ascontiguousarray(WvT[:, cols]))
        halves.append({
            "WqT_h": wq_h, "WqT_l": wq_l,
            "WkT_h": wk_h, "WkT_l": wk_l,
            "WvT_h": wv_h, "WvT_l": wv_l,
            # WoR[p, half, hp2, f] = Wo.T[hh*512 + (half*2+hp2)*128 + p, f]
            "WoR": np.ascontiguousarray(
                WoT[cols].reshape(2, 2, 128, D).transpose(2, 0, 1, 3).astype(bf16)),
            # q/k biases carry the WS weight scale (folded out in exp scale)
            "bq_pf": np.ascontiguousarray(bq[cols].reshape(FC, 128).T) * WS,
            "bk_pf": np.ascontiguousarray(bk[cols].reshape(FC, 128).T) * WS,
        })

    in_maps = []
    for c in range(NC):
        b, hh = c // 2, c % 2
        m = {
            "qT_h": xT["q"][b][0], "qT_l": xT["q"][b][1],
            "kT_h": xT["k"][b][0], "kT_l": xT["k"][b][1],
            "vT_h": xT["v"][b][0], "vT_l": xT["v"][b][1],
        }
        m.update(halves[hh])
        in_maps.append(m)

    res = run_bass_kernel_spmd(nc, in_maps, core_ids=list(range(NC)))

    # pairwise all-reduce of the tensor-parallel output projection (unshard).
    # bo and the bv contribution (bv @ Wo.T, constant across positions) are
    # folded into the host-side reduction.
    bias_row = bo + bv @ Wo.T
    y = np.empty((B, S, D), dtype=np.float32)
    for b in range(B):
        np.add(res.results[2 * b]["y_out"], res.results[2 * b + 1]["y_out"], out=y[b])
        y[b] += bias_row
    return y
